# revision 11
# baseline (speedup 1.0000x reference)
"""Causal multi-head self-attention on 8 Trainium2 NeuronCores.

Sharding: 8 cores = (batch b in 0..3) x (head-half hh in 0..1).
Each core computes Q/K/V projections for its 1024-wide slice of the hidden
dim (8 of 16 heads), causal attention for those heads, and the partial
output projection against the matching 1024 rows of Wo^T.  The two partial
outputs per batch are summed on the host at gather time (the tensor-parallel
all-reduce).

All matmuls run in float32r (full-rate fp32 path on the PE, ~1 cyc/row at
N>=256).  Attention is processed in q-tile pairs with the output projection
for the finished pair interleaved between pairs, which keeps TensorE densely
busy (HAM stays un-throttled) and overlaps softmax (DVE/ACT) with matmuls.

Problem constants (hardcoded): B=4, S=1024, D=2048, H=16, DH=128,
softmax scale = sqrt(DH) (faithful to the reference, which multiplies
scores by head_dim**0.5).
"""

import numpy as np

import concourse.bass as bass
import concourse.tile as tile
from concourse import bacc, mybir
from concourse.bass_utils import run_bass_kernel_spmd

B, S, D = 4, 1024, 2048
H = 16
DH = 128
SCALE = float(DH) ** 0.5
HL = 8          # heads per core
E = HL * DH     # 1024: per-core slice of hidden dim
KO = D // 128   # 16 k-subtiles for d-contraction
ST = S // 128   # 8 sequence tiles
F32 = mybir.dt.float32
F32R = mybir.dt.float32r
NEG = -1.0e30


def build_nc():
    nc = bacc.Bacc("TRN2", target_bir_lowering=False, debug=False, num_devices=8)

    xt = nc.dram_tensor("xt", [D, S], F32R, kind="ExternalInput")       # x[b].T
    wqt = nc.dram_tensor("wqt", [D, E], F32R, kind="ExternalInput")     # Wq[slice].T
    wkt = nc.dram_tensor("wkt", [D, E], F32R, kind="ExternalInput")
    wvt = nc.dram_tensor("wvt", [D, E], F32R, kind="ExternalInput")
    wot = nc.dram_tensor("wot", [E, D], F32R, kind="ExternalInput")     # Wo[:, slice].T
    maskd = nc.dram_tensor("maskd", [128, 128], F32, kind="ExternalInput")
    identd = nc.dram_tensor("identd", [128, 128], F32R, kind="ExternalInput")
    out = nc.dram_tensor("out", [S, D], F32, kind="ExternalOutput")

    xt_r = xt.ap().rearrange("(ko p) s -> p ko s", p=128)    # [128, 16, 1024]
    wqt_r = wqt.ap().rearrange("(ko p) e -> p ko e", p=128)
    wkt_r = wkt.ap().rearrange("(ko p) e -> p ko e", p=128)
    wvt_r = wvt.ap().rearrange("(ko p) e -> p ko e", p=128)
    wot_r = wot.ap().rearrange("(eo p) o -> p eo o", p=128)  # [128, 8, 2048]
    out_r = out.ap().rearrange("(so p) o -> p so o", p=128)  # [128, 8, 2048]

    with tile.TileContext(nc) as tc:
        # PSUM pools: 8 banks total on the core.
        pp = tc.alloc_tile_pool(name="pp", bufs=2, space="PSUM")      # proj/outproj
        ps_s = tc.alloc_tile_pool(name="ps_s", bufs=2, space="PSUM")  # scores
        ps_t = tc.alloc_tile_pool(name="ps_t", bufs=2, space="PSUM")  # transposes
        ps_c = tc.alloc_tile_pool(name="ps_c", bufs=2, space="PSUM")  # ctx accum

        # Long-lived SBUF (stack-allocated first).
        persist = tc.alloc_tile_pool(name="persist", bufs=1)
        mask_sb = persist.tile([128, 128], F32)
        ident_sb = persist.tile([128, 128], F32R)
        nc.sync.dma_start(mask_sb[:], maskd.ap())
        nc.sync.dma_start(ident_sb[:], identd.ap())
        qt_sb = persist.tile([128, HL, S], F32R)   # QT: [dh, head, s]
        kt_sb = persist.tile([128, HL, S], F32R)
        v_sb = persist.tile([128, ST, E], F32R)    # V: [s_in, s_out, e]
        ppool = tc.alloc_tile_pool(name="ppool", bufs=4)    # softmax P rows
        smalls = tc.alloc_tile_pool(name="smalls", bufs=8)  # per-row stats

        # ---------------- Phase 1: projections ----------------
        xpool = tc.alloc_tile_pool(name="xpool", bufs=1)
        xT_sb = xpool.tile([128, KO, S], F32R)

        wqk = tc.alloc_tile_pool(name="wqk", bufs=3)
        # First weight tile before the bulk x loads so the PE can start early.
        wt_first = wqk.tile([128, KO, 128], F32R, tag="wqk")
        nc.sync.dma_start(wt_first[:], wqt_r[:, :, 0:128])
        for kc in range(4):
            nc.sync.dma_start(
                xT_sb[:, 4 * kc:4 * (kc + 1), :], xt_r[:, 4 * kc:4 * (kc + 1), :]
            )

        # QT / KT: [e, s] = W_slice @ x^T  (lhsT = w tile, rhs = xT)
        for wsrc, dst in ((wqt_r, qt_sb), (wkt_r, kt_sb)):
            for e in range(HL):
                if wsrc is wqt_r and e == 0:
                    wt = wt_first
                else:
                    wt = wqk.tile([128, KO, 128], F32R, tag="wqk")
                    nc.sync.dma_start(wt[:], wsrc[:, :, e * 128:(e + 1) * 128])
                for sc in range(2):
                    ps = pp.tile([128, 512], F32, tag="pp")
                    for k in range(KO):
                        nc.tensor.matmul(
                            ps[:],
                            wt[:, k, :],
                            xT_sb[:, k, sc * 512:(sc + 1) * 512],
                            start=(k == 0),
                            stop=(k == KO - 1),
                        )
                    nc.scalar.copy(dst[:, e, sc * 512:(sc + 1) * 512], ps[:])
        wqk.release()

        # V: [s, e] = x @ Wv_slice^T  (lhsT = xT tile, rhs = wv tile)
        wv = tc.alloc_tile_pool(name="wv", bufs=3)
        for ec in range(4):
            wts = []
            for kg in range(2):
                w = wv.tile([128, 8, 256], F32R, tag="wv")
                nc.sync.dma_start(
                    w[:], wvt_r[:, 8 * kg:8 * (kg + 1), ec * 256:(ec + 1) * 256]
                )
                wts.append(w)
            for si in range(ST):
                ps = pp.tile([128, 512], F32, tag="pp")
                for k in range(KO):
                    nc.tensor.matmul(
                        ps[:, :256],
                        xT_sb[:, k, si * 128:(si + 1) * 128],
                        wts[k // 8][:, k % 8, :],
                        start=(k == 0),
                        stop=(k == KO - 1),
                    )
                nc.scalar.copy(v_sb[:, si, ec * 256:(ec + 1) * 256], ps[:, :256])
        wv.release()
        xpool.release()

        # ---------------- Phase 2: attention + interleaved out-proj --------
        # These pools reuse the released xT/weight-stream address range; their
        # first writes wait on the tail of the V projection, which is fine
        # because PV consumes V anyway.
        ptpool = tc.alloc_tile_pool(name="ptpool", bufs=6)   # transposed P
        ctxpp = tc.alloc_tile_pool(name="ctxpp", bufs=2)     # per-pair ctx^T
        stage = tc.alloc_tile_pool(name="stage", bufs=3)     # out staging
        wo = tc.alloc_tile_pool(name="wo", bufs=4)
        wo_tiles = []
        for oc in range(4):
            w = wo.tile([128, HL, 512], F32R, tag="wo")
            nc.sync.dma_start(w[:], wot_r[:, :, oc * 512:(oc + 1) * 512])
            wo_tiles.append(w)

        def scores_softmax(h, t):
            """Masked scaled softmax row block for (head h, q-tile t) -> p tile."""
            width = (t + 1) * 128
            c0w = min(width, 512)
            c1w = width - c0w
            qs = qt_sb[:, h, t * 128:(t + 1) * 128]
            ps0 = ps_s.tile([128, 512], F32, tag="ps_s")
            nc.tensor.matmul(
                ps0[:, :c0w], qs, kt_sb[:, h, 0:c0w], start=True, stop=True
            )
            ps1 = None
            if c1w:
                ps1 = ps_s.tile([128, 512], F32, tag="ps_s")
                nc.tensor.matmul(
                    ps1[:, :c1w], qs, kt_sb[:, h, 512:512 + c1w],
                    start=True, stop=True,
                )
            # causal mask on the diagonal 128x128 block
            if t < 4:
                diag = ps0[:, t * 128:(t + 1) * 128]
            else:
                diag = ps1[:, (t - 4) * 128:(t - 3) * 128]
            nc.vector.tensor_add(diag, diag, mask_sb[:])

            nm = smalls.tile([128, 1], F32, tag="nm")
            nc.vector.reduce_max(
                nm[:], ps0[:, :c0w], axis=mybir.AxisListType.X, negate=True
            )
            if c1w:
                nm1 = smalls.tile([128, 1], F32, tag="nm1")
                nc.vector.reduce_max(
                    nm1[:], ps1[:, :c1w], axis=mybir.AxisListType.X, negate=True
                )
                nc.vector.tensor_tensor(nm[:], nm[:], nm1[:], mybir.AluOpType.min)
            bias = smalls.tile([128, 1], F32, tag="bias")
            nc.vector.tensor_scalar_mul(bias[:], nm[:], SCALE)

            p_sb = ppool.tile([128, S], F32R, tag="p")
            r0 = smalls.tile([128, 1], F32, tag="r0")
            nc.scalar.activation(
                p_sb[:, :c0w], ps0[:, :c0w], mybir.ActivationFunctionType.Exp,
                bias=bias[:], scale=SCALE, accum_out=r0[:],
            )
            if c1w:
                r1 = smalls.tile([128, 1], F32, tag="r1")
                nc.scalar.activation(
                    p_sb[:, 512:512 + c1w], ps1[:, :c1w],
                    mybir.ActivationFunctionType.Exp,
                    bias=bias[:], scale=SCALE, accum_out=r1[:],
                )
                nc.vector.tensor_add(r0[:], r0[:], r1[:])
            rr = smalls.tile([128, 1], F32, tag="rr")
            nc.vector.reciprocal(rr[:], r0[:])
            # normalization runs on the otherwise-idle GpSimd engine
            nc.gpsimd.tensor_scalar_mul(p_sb[:, :width], p_sb[:, :width], rr[:])
            return p_sb

        def outproj_chunk(ctx_tile, tp, chunk):
            """One of the 8 output-projection chunks for finished pair tp."""
            sl, oc = divmod(chunk, 4)
            si = 2 * tp + sl
            ps = pp.tile([128, 512], F32, tag="pp")
            for j in range(HL):
                nc.tensor.matmul(
                    ps[:],
                    ctx_tile[:, j, sl * 128:(sl + 1) * 128],
                    wo_tiles[oc][:, j, :],
                    start=(j == 0),
                    stop=(j == HL - 1),
                )
            ob = stage.tile([128, 512], F32, tag="ob")
            nc.scalar.copy(ob[:], ps[:])
            nc.sync.dma_start(out_r[:, si, oc * 512:(oc + 1) * 512], ob[:])

        pending = None  # (ctx_tile, tp) whose out-proj interleaves with next pair
        for tp in range(4):
            ctx_pair = ctxpp.tile([128, HL, 256], F32R, tag="ctx")
            nk = 2 * tp + 2
            for h in range(HL):
                pe_ = scores_softmax(h, 2 * tp)
                po_ = scores_softmax(h, 2 * tp + 1)
                ctx_ps = ps_c.tile([128, 256], F32, tag="ps_c")
                for j in range(nk):
                    pt_ps = ps_t.tile([128, 256], F32R, tag="ps_t")
                    pt_sb = ptpool.tile([128, 256], F32R, tag="pt")
                    last = j == nk - 1
                    cp = nc.vector.tensor_copy if j % 2 else nc.scalar.copy
                    if not last:
                        nc.tensor.transpose(
                            pt_ps[:, 0:128], pe_[:, j * 128:(j + 1) * 128],
                            ident_sb[:],
                        )
                    nc.tensor.transpose(
                        pt_ps[:, 128:256], po_[:, j * 128:(j + 1) * 128],
                        ident_sb[:],
                    )
                    vt = v_sb[:, j, h * 128:(h + 1) * 128]
                    if not last:
                        cp(pt_sb[:], pt_ps[:])
                        nc.tensor.matmul(
                            ctx_ps[:], vt, pt_sb[:], start=(j == 0), stop=last
                        )
                    else:
                        cp(pt_sb[:, 128:256], pt_ps[:, 128:256])
                        nc.tensor.matmul(
                            ctx_ps[:, 128:256], vt, pt_sb[:, 128:256],
                            start=(j == 0), stop=True,
                        )
                nc.scalar.copy(ctx_pair[:, h, :], ctx_ps[:])
                # spread the previous pair's out-proj through this pair's
                # attention so TensorE keeps seeing dense normal matmuls
                if pending is not None:
                    outproj_chunk(pending[0], pending[1], h)
            pending = (ctx_pair, tp)
        for chunk in range(8):
            outproj_chunk(pending[0], pending[1], chunk)

        for p in (wo, stage, ctxpp, ptpool, smalls, ppool, persist, ps_c, ps_t, ps_s, pp):
            p.release()

    nc.compile()
    return nc


_NC = None


def _get_nc():
    global _NC
    if _NC is None:
        _NC = build_nc()
    return _NC


def _make_in_maps(x, Wq, Wk, Wv, Wo):
    x = np.asarray(x, dtype=np.float32)
    Wq = np.asarray(Wq, dtype=np.float32)
    Wk = np.asarray(Wk, dtype=np.float32)
    Wv = np.asarray(Wv, dtype=np.float32)
    Wo = np.asarray(Wo, dtype=np.float32)

    mask = np.triu(np.full((128, 128), NEG, dtype=np.float32), k=1)
    ident = np.eye(128, dtype=np.float32)

    xts = [np.ascontiguousarray(x[b].T) for b in range(B)]
    wqts = [np.ascontiguousarray(Wq[hh * E:(hh + 1) * E, :].T) for hh in range(2)]
    wkts = [np.ascontiguousarray(Wk[hh * E:(hh + 1) * E, :].T) for hh in range(2)]
    wvts = [np.ascontiguousarray(Wv[hh * E:(hh + 1) * E, :].T) for hh in range(2)]
    wots = [np.ascontiguousarray(Wo[:, hh * E:(hh + 1) * E].T) for hh in range(2)]

    in_maps = []
    for b in range(B):
        for hh in range(2):
            in_maps.append({
                "xt": xts[b],
                "wqt": wqts[hh],
                "wkt": wkts[hh],
                "wvt": wvts[hh],
                "wot": wots[hh],
                "maskd": mask,
                "identd": ident,
            })
    return in_maps


def run(x, Wq, Wk, Wv, Wo, **rb_kwargs):
    """Run on 8 cores; returns (output [B,S,D], BassKernelResults)."""
    nc = _get_nc()
    in_maps = _make_in_maps(x, Wq, Wk, Wv, Wo)
    res = run_bass_kernel_spmd(nc, in_maps, core_ids=list(range(8)), **rb_kwargs)
    out = np.empty((B, S, D), dtype=np.float32)
    for b in range(B):
        out[b] = res.results[2 * b]["out"] + res.results[2 * b + 1]["out"]
    return out, res


def kernel(x, Wq, Wk, Wv, Wo):
    out, _ = run(x, Wq, Wk, Wv, Wo)
    return out


# revision 12
# speedup vs baseline: 1.9877x; 1.9877x over previous
"""Causal multi-head self-attention on 8 Trainium2 NeuronCores.

Sharding: 8 cores = (batch b in 0..3) x (head-half hh in 0..1).
Each core computes Q/K/V projections for its 1024-wide slice of the hidden
dim (8 of 16 heads), causal attention for those heads, and the partial
output projection against the matching 1024 rows of Wo^T.  The two partial
outputs per batch are summed on the host at gather time (the tensor-parallel
all-reduce).

All matmuls run in float32r (full-rate fp32 path on the PE, ~1 cyc/row at
N>=256).  Attention is processed in q-tile pairs with the output projection
for the finished pair interleaved between pairs, which keeps TensorE densely
busy (HAM stays un-throttled) and overlaps softmax (DVE/ACT) with matmuls.

Problem constants (hardcoded): B=4, S=1024, D=2048, H=16, DH=128,
softmax scale = sqrt(DH) (faithful to the reference, which multiplies
scores by head_dim**0.5).
"""

import numpy as np

import concourse.bass as bass
import concourse.tile as tile
from concourse import bacc, mybir
from concourse.bass_utils import run_bass_kernel_spmd

B, S, D = 4, 1024, 2048
H = 16
DH = 128
SCALE = float(DH) ** 0.5
HL = 8          # heads per core
E = HL * DH     # 1024: per-core slice of hidden dim
KO = D // 128   # 16 k-subtiles for d-contraction
ST = S // 128   # 8 sequence tiles
F32 = mybir.dt.float32
F32R = mybir.dt.float32r
NEG = -1.0e30


def build_nc():
    nc = bacc.Bacc("TRN2", target_bir_lowering=False, debug=False, num_devices=8)

    xt = nc.dram_tensor("xt", [D, S], F32R, kind="ExternalInput")       # x[b].T
    wqt = nc.dram_tensor("wqt", [D, E], F32R, kind="ExternalInput")     # Wq[slice].T
    wkt = nc.dram_tensor("wkt", [D, E], F32R, kind="ExternalInput")
    wvt = nc.dram_tensor("wvt", [D, E], F32R, kind="ExternalInput")
    wot = nc.dram_tensor("wot", [E, D], F32R, kind="ExternalInput")     # Wo[:, slice].T
    maskd = nc.dram_tensor("maskd", [128, 128], F32, kind="ExternalInput")
    identd = nc.dram_tensor("identd", [128, 128], F32R, kind="ExternalInput")
    out = nc.dram_tensor("out", [S, D], F32, kind="ExternalOutput")

    xt_r = xt.ap().rearrange("(ko p) s -> p ko s", p=128)    # [128, 16, 1024]
    wqt_r = wqt.ap().rearrange("(ko p) e -> p ko e", p=128)
    wkt_r = wkt.ap().rearrange("(ko p) e -> p ko e", p=128)
    wvt_r = wvt.ap().rearrange("(ko p) e -> p ko e", p=128)
    wot_r = wot.ap().rearrange("(eo p) o -> p eo o", p=128)  # [128, 8, 2048]
    out_r = out.ap().rearrange("(so p) o -> p so o", p=128)  # [128, 8, 2048]

    with tile.TileContext(nc) as tc:
        # PSUM pools: 8 banks total on the core.
        pp = tc.alloc_tile_pool(name="pp", bufs=2, space="PSUM")      # proj/outproj
        ps_s = tc.alloc_tile_pool(name="ps_s", bufs=2, space="PSUM")  # scores
        ps_t = tc.alloc_tile_pool(name="ps_t", bufs=2, space="PSUM")  # transposes
        ps_c = tc.alloc_tile_pool(name="ps_c", bufs=2, space="PSUM")  # ctx accum

        # Long-lived SBUF (stack-allocated first).
        persist = tc.alloc_tile_pool(name="persist", bufs=1)
        mask_sb = persist.tile([128, 128], F32)
        ident_sb = persist.tile([128, 128], F32R)
        nc.sync.dma_start(mask_sb[:], maskd.ap())
        nc.sync.dma_start(ident_sb[:], identd.ap())
        qt_sb = persist.tile([128, HL, S], F32R)   # QT: [dh, head, s]
        kt_sb = persist.tile([128, HL, S], F32R)
        v_sb = persist.tile([128, ST, E], F32R)    # V: [s_in, s_out, e]
        ppool = tc.alloc_tile_pool(name="ppool", bufs=4)    # softmax P rows
        smalls = tc.alloc_tile_pool(name="smalls", bufs=8)  # per-row stats

        # ---------------- Phase 1: projections ----------------
        xpool = tc.alloc_tile_pool(name="xpool", bufs=1)
        xT_sb = xpool.tile([128, KO, S], F32R)

        wqk = tc.alloc_tile_pool(name="wqk", bufs=3)
        # First weight tile before the bulk x loads so the PE can start early.
        wt_first = wqk.tile([128, KO, 128], F32R, tag="wqk")
        nc.sync.dma_start(wt_first[:], wqt_r[:, :, 0:128])
        for kc in range(4):
            nc.sync.dma_start(
                xT_sb[:, 4 * kc:4 * (kc + 1), :], xt_r[:, 4 * kc:4 * (kc + 1), :]
            )

        # QT / KT: [e, s] = W_slice @ x^T  (lhsT = w tile, rhs = xT)
        for wsrc, dst in ((wqt_r, qt_sb), (wkt_r, kt_sb)):
            for e in range(HL):
                if wsrc is wqt_r and e == 0:
                    wt = wt_first
                else:
                    wt = wqk.tile([128, KO, 128], F32R, tag="wqk")
                    nc.sync.dma_start(wt[:], wsrc[:, :, e * 128:(e + 1) * 128])
                for sc in range(2):
                    ps = pp.tile([128, 512], F32, tag="pp")
                    for k in range(KO):
                        nc.tensor.matmul(
                            ps[:],
                            wt[:, k, :],
                            xT_sb[:, k, sc * 512:(sc + 1) * 512],
                            start=(k == 0),
                            stop=(k == KO - 1),
                        )
                    nc.scalar.copy(dst[:, e, sc * 512:(sc + 1) * 512], ps[:])
        wqk.release()

        # V: [s, e] = x @ Wv_slice^T  (lhsT = xT tile, rhs = wv tile)
        wv = tc.alloc_tile_pool(name="wv", bufs=3)
        for ec in range(4):
            wts = []
            for kg in range(2):
                w = wv.tile([128, 8, 256], F32R, tag="wv")
                nc.sync.dma_start(
                    w[:], wvt_r[:, 8 * kg:8 * (kg + 1), ec * 256:(ec + 1) * 256]
                )
                wts.append(w)
            for si in range(ST):
                ps = pp.tile([128, 512], F32, tag="pp")
                for k in range(KO):
                    nc.tensor.matmul(
                        ps[:, :256],
                        xT_sb[:, k, si * 128:(si + 1) * 128],
                        wts[k // 8][:, k % 8, :],
                        start=(k == 0),
                        stop=(k == KO - 1),
                    )
                nc.scalar.copy(v_sb[:, si, ec * 256:(ec + 1) * 256], ps[:, :256])
        wv.release()
        xpool.release()

        # ---------------- Phase 2: attention + interleaved out-proj --------
        # These pools reuse the released xT/weight-stream address range; their
        # first writes wait on the tail of the V projection, which is fine
        # because PV consumes V anyway.
        ptpool = tc.alloc_tile_pool(name="ptpool", bufs=6)   # transposed P
        ctxpp = tc.alloc_tile_pool(name="ctxpp", bufs=2)     # per-pair ctx^T
        stage = tc.alloc_tile_pool(name="stage", bufs=3)     # out staging
        wo = tc.alloc_tile_pool(name="wo", bufs=4)
        wo_tiles = []
        for oc in range(4):
            w = wo.tile([128, HL, 512], F32R, tag="wo")
            nc.sync.dma_start(w[:], wot_r[:, :, oc * 512:(oc + 1) * 512])
            wo_tiles.append(w)

        def scores_softmax(h, t):
            """Masked scaled softmax row block for (head h, q-tile t) -> p tile."""
            width = (t + 1) * 128
            c0w = min(width, 512)
            c1w = width - c0w
            qs = qt_sb[:, h, t * 128:(t + 1) * 128]
            ps0 = ps_s.tile([128, 512], F32, tag="ps_s")
            nc.tensor.matmul(
                ps0[:, :c0w], qs, kt_sb[:, h, 0:c0w], start=True, stop=True
            )
            ps1 = None
            if c1w:
                ps1 = ps_s.tile([128, 512], F32, tag="ps_s")
                nc.tensor.matmul(
                    ps1[:, :c1w], qs, kt_sb[:, h, 512:512 + c1w],
                    start=True, stop=True,
                )
            # causal mask on the diagonal 128x128 block
            if t < 4:
                diag = ps0[:, t * 128:(t + 1) * 128]
            else:
                diag = ps1[:, (t - 4) * 128:(t - 3) * 128]
            nc.vector.tensor_add(diag, diag, mask_sb[:])

            nm = smalls.tile([128, 1], F32, tag="nm")
            nc.vector.reduce_max(
                nm[:], ps0[:, :c0w], axis=mybir.AxisListType.X, negate=True
            )
            if c1w:
                nm1 = smalls.tile([128, 1], F32, tag="nm1")
                nc.vector.reduce_max(
                    nm1[:], ps1[:, :c1w], axis=mybir.AxisListType.X, negate=True
                )
                nc.vector.tensor_tensor(nm[:], nm[:], nm1[:], mybir.AluOpType.min)
            bias = smalls.tile([128, 1], F32, tag="bias")
            nc.vector.tensor_scalar_mul(bias[:], nm[:], SCALE)

            p_sb = ppool.tile([128, S], F32R, tag="p")
            r0 = smalls.tile([128, 1], F32, tag="r0")
            nc.scalar.activation(
                p_sb[:, :c0w], ps0[:, :c0w], mybir.ActivationFunctionType.Exp,
                bias=bias[:], scale=SCALE, accum_out=r0[:],
            )
            if c1w:
                r1 = smalls.tile([128, 1], F32, tag="r1")
                nc.scalar.activation(
                    p_sb[:, 512:512 + c1w], ps1[:, :c1w],
                    mybir.ActivationFunctionType.Exp,
                    bias=bias[:], scale=SCALE, accum_out=r1[:],
                )
                nc.vector.tensor_add(r0[:], r0[:], r1[:])
            rr = smalls.tile([128, 1], F32, tag="rr")
            nc.vector.reciprocal(rr[:], r0[:])
            nc.vector.tensor_scalar_mul(p_sb[:, :width], p_sb[:, :width], rr[:])
            return p_sb

        def outproj_chunk(ctx_tile, tp, chunk):
            """One of the 8 output-projection chunks for finished pair tp."""
            sl, oc = divmod(chunk, 4)
            si = 2 * tp + sl
            ps = pp.tile([128, 512], F32, tag="pp")
            for j in range(HL):
                nc.tensor.matmul(
                    ps[:],
                    ctx_tile[:, j, sl * 128:(sl + 1) * 128],
                    wo_tiles[oc][:, j, :],
                    start=(j == 0),
                    stop=(j == HL - 1),
                )
            ob = stage.tile([128, 512], F32, tag="ob")
            nc.scalar.copy(ob[:], ps[:])
            nc.sync.dma_start(out_r[:, si, oc * 512:(oc + 1) * 512], ob[:])

        pending = None  # (ctx_tile, tp) whose out-proj interleaves with next pair
        for tp in range(4):
            ctx_pair = ctxpp.tile([128, HL, 256], F32R, tag="ctx")
            nk = 2 * tp + 2
            for h in range(HL):
                pe_ = scores_softmax(h, 2 * tp)
                po_ = scores_softmax(h, 2 * tp + 1)
                ctx_ps = ps_c.tile([128, 256], F32, tag="ps_c")
                for j in range(nk):
                    pt_ps = ps_t.tile([128, 256], F32R, tag="ps_t")
                    pt_sb = ptpool.tile([128, 256], F32R, tag="pt")
                    last = j == nk - 1
                    cp = nc.vector.tensor_copy if j % 2 else nc.scalar.copy
                    if not last:
                        nc.tensor.transpose(
                            pt_ps[:, 0:128], pe_[:, j * 128:(j + 1) * 128],
                            ident_sb[:],
                        )
                    nc.tensor.transpose(
                        pt_ps[:, 128:256], po_[:, j * 128:(j + 1) * 128],
                        ident_sb[:],
                    )
                    vt = v_sb[:, j, h * 128:(h + 1) * 128]
                    if not last:
                        cp(pt_sb[:], pt_ps[:])
                        nc.tensor.matmul(
                            ctx_ps[:], vt, pt_sb[:], start=(j == 0), stop=last
                        )
                    else:
                        cp(pt_sb[:, 128:256], pt_ps[:, 128:256])
                        nc.tensor.matmul(
                            ctx_ps[:, 128:256], vt, pt_sb[:, 128:256],
                            start=(j == 0), stop=True,
                        )
                nc.scalar.copy(ctx_pair[:, h, :], ctx_ps[:])
                # spread the previous pair's out-proj through this pair's
                # attention so TensorE keeps seeing dense normal matmuls
                if pending is not None:
                    outproj_chunk(pending[0], pending[1], h)
            pending = (ctx_pair, tp)
        for chunk in range(8):
            outproj_chunk(pending[0], pending[1], chunk)

        for p in (wo, stage, ctxpp, ptpool, smalls, ppool, persist, ps_c, ps_t, ps_s, pp):
            p.release()

    nc.compile()
    return nc


_NC = None


def _get_nc():
    global _NC
    if _NC is None:
        _NC = build_nc()
    return _NC


def _make_in_maps(x, Wq, Wk, Wv, Wo):
    x = np.asarray(x, dtype=np.float32)
    Wq = np.asarray(Wq, dtype=np.float32)
    Wk = np.asarray(Wk, dtype=np.float32)
    Wv = np.asarray(Wv, dtype=np.float32)
    Wo = np.asarray(Wo, dtype=np.float32)

    mask = np.triu(np.full((128, 128), NEG, dtype=np.float32), k=1)
    ident = np.eye(128, dtype=np.float32)

    xts = [np.ascontiguousarray(x[b].T) for b in range(B)]
    wqts = [np.ascontiguousarray(Wq[hh * E:(hh + 1) * E, :].T) for hh in range(2)]
    wkts = [np.ascontiguousarray(Wk[hh * E:(hh + 1) * E, :].T) for hh in range(2)]
    wvts = [np.ascontiguousarray(Wv[hh * E:(hh + 1) * E, :].T) for hh in range(2)]
    wots = [np.ascontiguousarray(Wo[:, hh * E:(hh + 1) * E].T) for hh in range(2)]

    in_maps = []
    for b in range(B):
        for hh in range(2):
            in_maps.append({
                "xt": xts[b],
                "wqt": wqts[hh],
                "wkt": wkts[hh],
                "wvt": wvts[hh],
                "wot": wots[hh],
                "maskd": mask,
                "identd": ident,
            })
    return in_maps


def run(x, Wq, Wk, Wv, Wo, **rb_kwargs):
    """Run on 8 cores; returns (output [B,S,D], BassKernelResults)."""
    nc = _get_nc()
    in_maps = _make_in_maps(x, Wq, Wk, Wv, Wo)
    res = run_bass_kernel_spmd(nc, in_maps, core_ids=list(range(8)), **rb_kwargs)
    out = np.empty((B, S, D), dtype=np.float32)
    for b in range(B):
        out[b] = res.results[2 * b]["out"] + res.results[2 * b + 1]["out"]
    return out, res


def kernel(x, Wq, Wk, Wv, Wo):
    out, _ = run(x, Wq, Wk, Wv, Wo)
    return out
